# revision 11
# baseline (speedup 1.0000x reference)
"""GATv2 x5 (gnn_message_passing) on 8 Trainium2 NeuronCores.

Sharding: nodes partitioned across 8 cores by destination-node owner
(6250 nodes/core, padded to 6272 = 49 tiles of 128). Edges live with
their dst owner, grouped into 128-edge chunks per dst-tile, split by
src half (local row < 4096 vs >= 4096) so the two per-layer AllGathers
can overlap edge compute.

v2 changes vs the gather-everything baseline:
- xr[dst] rows are no longer DMA-gathered per edge. A static one-hot
  matrix OneHotS[s, e] (s = dst slot in tile, e = edge slot) is stored
  in DRAM (bf16) and loaded linearly per supertile; XR for a chunk is
  produced on the tensor engine as OneHotS^T @ xr_tile, accumulated in
  PSUM together with an identity-matmul of the gathered XL chunk, so
  S = XL + XR materializes directly in PSUM with no vector add.
- attention vector folded into the weights: Wl/Wr columns are permuted
  (a >= 0 first, p columns) and scaled by |a| (host side). The GATv2
  score becomes a plain column-sum of a per-sign Prelu: pos cols
  Prelu(alpha=.2), neg cols Prelu(alpha=5, scale=-.2). Scalar engine
  computes the activations (batched over 8-chunk sub-batches straight
  from PSUM), vector does one 3D tensor_reduce per sub-batch. The |a|
  scaling is undone by folding 1/|a| into the next layer's weight rows
  (final layer: on the host).
- the scaled one-hot Oc (softmax numerator scatter matrix) is built in
  two batched DVE ops per sub-batch using stride-0 broadcast APs
  (is_equal against iota, then multiply by exp(score)), replacing the
  per-chunk tensor_scalar ops that dominated DVE time.
All matmuls and gathers in bf16; accumulation, scores in fp32.
"""
import sys
import numpy as np

sys.path.insert(0, "/opt/trn_rl_repo")

import concourse.bass as bass
import concourse.bacc as bacc
import concourse.mybir as mybir
import concourse.tile as tile
from concourse.bass_utils import run_bass_kernel_spmd
from concourse.masks import make_identity

F32 = mybir.dt.float32
BF16 = mybir.dt.bfloat16
I16 = mybir.dt.int16
AF = mybir.ActivationFunctionType
OP = mybir.AluOpType
AX = mybir.AxisListType

N = 50000
DIN = 7
D = 128
T = 5
CORES = 8
SH = N // CORES            # 6250 nodes per core
TILES = 49
SHP = TILES * 128          # 6272 padded nodes per core
NP_ALL = CORES * SHP       # 50176 slots globally
H1 = 4096                  # local rows [0, 4096) -> AllGather half 1
H2 = SHP - H1              # 2176 rows           -> half 2 (8*H1 = 32768
                           # so half-1 gather rows exactly fit int16)
NEG = 0.2
G = 8                      # chunks per PSUM sub-batch


def _build_nc(params):
    """params = (K1s, K2s, ps): per-dst-tile chunk counts for src-half 1/2
    and per-layer positive-attention column counts."""
    K1s, K2s, ps = params
    CH = sum(K1s) + sum(K2s)   # total chunks per core

    # supertiles: pairs of dst tiles processed together
    groups = [(t, t + 1) for t in range(0, TILES - 1, 2)] + [(TILES - 1,)]
    self_qn = [0]   # round-robin SWDGE queue assignment

    nc = bacc.Bacc("TRN2", target_bir_lowering=False, debug=False,
                   num_devices=CORES, num_swdge_queues=4)

    xT_own = nc.dram_tensor("xT_own", [DIN, SHP], BF16, kind="ExternalInput")
    Wlr0 = nc.dram_tensor("Wlr0", [DIN, 2 * D], BF16, kind="ExternalInput")
    Wlr = nc.dram_tensor("Wlr", [T - 1, D, 2 * D], BF16, kind="ExternalInput")
    br2 = nc.dram_tensor("br2", [T, D], F32, kind="ExternalInput")
    bout = nc.dram_tensor("bout", [D, T], F32, kind="ExternalInput")
    ixl_i = nc.dram_tensor("ixl", [128, CH * 8], I16, kind="ExternalInput")
    hs_i = nc.dram_tensor("hs", [128, CH * 128], BF16, kind="ExternalInput")
    he_i = nc.dram_tensor("he", [128, CH * 128], BF16, kind="ExternalInput")

    out_t = nc.dram_tensor("out", [SHP, D], F32, kind="ExternalOutput")

    with tile.TileContext(nc) as tc:
        with (
            tc.tile_pool(name="pers", bufs=1) as pers,
            tc.tile_pool(name="wl", bufs=2) as wl,
            tc.tile_pool(name="edge", bufs=2) as ep,
            tc.tile_pool(name="sb", bufs=3) as sbp,
            tc.tile_pool(name="pro", bufs=1, space="PSUM") as psp,
            tc.tile_pool(name="spsum", bufs=2, space="PSUM") as psS,
            tc.tile_pool(name="aggn", bufs=2, space="PSUM") as psan,
            tc.tile_pool(name="tr", bufs=1, space="PSUM") as pst,
            tc.tile_pool(name="dram", bufs=2, space="DRAM") as dp,
        ):
            # --- persistent setup ---
            ident = pers.tile([128, 128], BF16)
            make_identity(nc, ident[:])
            ixl_sb = pers.tile([128, CH * 8], I16)
            nc.sync.dma_start(out=ixl_sb[:], in_=ixl_i[:, :])
            xT_sb = pers.tile([DIN, SHP], BF16)
            nc.sync.dma_start(out=xT_sb[:], in_=xT_own[:, :])
            ones_c = pers.tile([128, 1], BF16)
            nc.vector.memset(ones_c[:], 1.0)
            # per-tile hidden-state tiles: precise deps let next-layer
            # prologue (and its AllGather) overlap this layer's edge tail
            hT_t = [pers.tile([128, 128], BF16, name=f"hT{t}")
                    for t in range(TILES)]

            lw = {}    # layer -> weight/bias tiles
            ldr = {}   # layer -> dram staging tiles

            def load_weights(l):
                d = {}
                w_sb = wl.tile([128, 2 * D], BF16, tag="w", name=f"w{l}")
                if l == 0:
                    nc.sync.dma_start(out=w_sb[:DIN, :], in_=Wlr0[:, :])
                else:
                    nc.sync.dma_start(out=w_sb[:], in_=Wlr[l - 1, :, :])
                d["w"] = w_sb
                br2_b = wl.tile([128, D], F32, tag="br2", name=f"br2{l}")
                nc.sync.dma_start(
                    out=br2_b[:],
                    in_=br2[l : l + 1, :].partition_broadcast(128))
                d["br2"] = br2_b
                bo_col = wl.tile([128, 1], F32, tag="boc", name=f"boc{l}")
                nc.sync.dma_start(out=bo_col[:], in_=bout[:, l : l + 1])
                d["boc"] = bo_col
                lw[l] = d
                ldr[l] = dict(
                    xl_cc=dp.tile([SHP, D], BF16, tag="xlcc",
                                  name=f"xlcc{l}"),
                    xr_dr=dp.tile([SHP, D], BF16, tag="xrdr",
                                  name=f"xrdr{l}"),
                )

            def prologue_tile(l, m):
                d = lw[l]
                dr = ldr[l]
                ps2 = psp.tile([128, 2 * D], F32, space="PSUM", tag="pro",
                               name=f"pro{l}_{m}")
                if l == 0:
                    lhsT = xT_sb[:, m * 128 : (m + 1) * 128]
                    rhs = d["w"][:DIN, :]
                else:
                    lhsT = hT_t[m][:]
                    rhs = d["w"][:, :]
                nc.tensor.matmul(out=ps2[:], lhsT=lhsT, rhs=rhs,
                                 start=True, stop=True)
                xl_sb = sbp.tile([128, D], BF16, tag="xls")
                nc.scalar.activation(out=xl_sb[:], in_=ps2[:, :D],
                                     func=AF.Identity)
                nc.sync.dma_start(
                    out=dr["xl_cc"][m * 128 : (m + 1) * 128, :], in_=xl_sb[:])
                xr_sb = sbp.tile([128, D], BF16, tag="xrs")
                nc.vector.tensor_tensor(
                    out=xr_sb[:], in0=ps2[:, D:], in1=d["br2"][:], op=OP.add)
                nc.sync.dma_start(
                    out=dr["xr_dr"][m * 128 : (m + 1) * 128, :], in_=xr_sb[:])

            def allgather(l, half):
                dr = ldr[l]
                if half == 0:
                    xl_h = dp.tile([CORES * H1, D], BF16, tag="xlh1",
                                   name=f"xlh1_{l}")
                    src = dr["xl_cc"][0:H1, :]
                    dr["h1"] = xl_h
                else:
                    xl_h = dp.tile([CORES * H2, D], BF16, tag="xlh2",
                                   name=f"xlh2_{l}")
                    src = dr["xl_cc"][H1:SHP, :]
                    dr["h2"] = xl_h
                nc.gpsimd.collective_compute(
                    "AllGather", OP.bypass,
                    replica_groups=[list(range(CORES))],
                    ins=[src.opt()],
                    outs=[xl_h[:, :].opt()],
                )

            # layer-0 prologue + gathers up front
            load_weights(0)
            for m in range(TILES):
                prologue_tile(0, m)
            allgather(0, 0)
            allgather(0, 1)

            for l in range(T):
                d_l = lw[l]
                bo_col = d_l["boc"]
                p_l = ps[l]
                xl_h1 = ldr[l]["h1"]
                xl_h2 = ldr[l]["h2"]
                xr_dr = ldr[l]["xr_dr"]
                if l < T - 1:
                    load_weights(l + 1)
                done_pro = 0

                # --- edge stage: per supertile (pair of dst tiles) ---
                pos = 0
                for ts in groups:
                    k1s = [K1s[t] for t in ts]
                    k2s = [K2s[t] for t in ts]
                    nA = sum(k1s)
                    nB = sum(k2s)
                    n = nA + nB
                    # chunk position -> (member index, dst-tile)
                    # layout: [A of ts[0] | A of ts[1] | B of ts[0] | B of ts[1]]
                    owner = []
                    for i, t in enumerate(ts):
                        owner += [i] * k1s[i]
                    for i, t in enumerate(ts):
                        owner += [i] * k2s[i]
                    first_chunk = {}
                    last_chunk = {}
                    for c, o in enumerate(owner):
                        if o not in first_chunk:
                            first_chunk[o] = c
                        last_chunk[o] = c

                    # dma_gather is capped at 1024 idxs (8 chunks of 128) by
                    # the SWDGE descriptor ring; split into sub-gathers spread
                    # over the 4 SWDGE queues.
                    def gathers(dst, src_ap, idx_sb, c0, nch):
                        off = 0
                        while off < nch:
                            g = min(8, nch - off)
                            nc.gpsimd.dma_gather(
                                dst[:, off : off + g, :], src_ap,
                                idx_sb[:, (c0 + off) * 8 : (c0 + off + g) * 8],
                                g * 128, g * 128, D,
                                queue_num=self_qn[0] % 4)
                            self_qn[0] += 1
                            off += g

                    XL = ep.tile([128, n, D], BF16, tag="XL", bufs=3)
                    if nA:
                        gathers(XL[:, :, :], xl_h1[:, :], ixl_sb, pos, nA)
                    if nB:
                        gathers(XL[:, nA:, :], xl_h2[:, :], ixl_sb,
                                pos + nA, nB)

                    # static one-hots (linear DMA) + xr rows of the 1-2 tiles
                    Hs = ep.tile([128, n, 128], BF16, tag="Hs", bufs=3)
                    nc.sync.dma_start(
                        out=Hs[:, :, :],
                        in_=hs_i[:, pos * 128 : (pos + n) * 128])
                    He = ep.tile([128, n, 128], BF16, tag="He", bufs=3)
                    nc.sync.dma_start(
                        out=He[:, :, :],
                        in_=he_i[:, pos * 128 : (pos + n) * 128])
                    xr_st = []
                    for i, t in enumerate(ts):
                        xr_i = sbp.tile([128, D], BF16, tag=f"xrst{i}")
                        nc.sync.dma_start(
                            out=xr_i[:],
                            in_=xr_dr[t * 128 : (t + 1) * 128, :])
                        xr_st.append(xr_i)

                    e_t = sbp.tile([128, n], F32, tag="e")
                    ex_t = sbp.tile([128, n], F32, tag="ex")
                    Oc = ep.tile([128, n, 128], BF16, tag="Oc")

                    ps_n = []
                    ps_d = []
                    for i in range(len(ts)):
                        pnd = psan.tile([128, D + 1], F32, space="PSUM",
                                        tag="aggn", name=f"psnd{i}")
                        ps_n.append(pnd[:, :D])
                        ps_d.append(pnd[:, D : D + 1])

                    for b0 in range(0, n, G):
                        g = min(G, n - b0)
                        S_ps = psS.tile([128, G, 128], F32, space="PSUM",
                                        tag="sps")
                        for j in range(g):
                            c = b0 + j
                            nc.tensor.matmul(
                                out=S_ps[:, j, :], lhsT=Hs[:, c, :],
                                rhs=xr_st[owner[c]][:],
                                start=True, stop=False,
                                skip_group_check=True)
                        # add gathered XL in 4-chunk (1 PSUM bank) slabs
                        for h0 in range(0, g, 4):
                            hh = min(4, g - h0)
                            nc.tensor.matmul(
                                out=S_ps[:, h0 : h0 + hh, :], lhsT=ident[:],
                                rhs=XL[:, b0 + h0 : b0 + h0 + hh, :],
                                start=False, stop=True,
                                skip_group_check=True)
                        # score: sign-folded prelu + column sum
                        Lpp = sbp.tile([128, G, 128], BF16, tag="lpp")
                        if p_l > 0:
                            nc.scalar.activation(
                                out=Lpp[:, :g, :p_l], in_=S_ps[:, :g, :p_l],
                                func=AF.Prelu, alpha=NEG)
                        if p_l < D:
                            nc.scalar.activation(
                                out=Lpp[:, :g, p_l:], in_=S_ps[:, :g, p_l:],
                                func=AF.Prelu, alpha=1.0 / NEG, scale=-NEG)
                        nc.vector.tensor_reduce(
                            out=e_t[:, b0 : b0 + g], in_=Lpp[:, :g, :],
                            axis=AX.X, op=OP.add)
                        nc.scalar.activation(
                            out=ex_t[:, b0 : b0 + g], in_=e_t[:, b0 : b0 + g],
                            func=AF.Exp)
                        # scaled one-hot Oc = OneHotE * ex (one-hot from DRAM)
                        nc.vector.tensor_tensor(
                            out=Oc[:, b0 : b0 + g, :],
                            in0=He[:, b0 : b0 + g, :],
                            in1=ex_t[:, b0 : b0 + g].unsqueeze(2)
                                .broadcast_to([128, g, 128]),
                            op=OP.mult)
                        for j in range(g):
                            c = b0 + j
                            o = owner[c]
                            st = c == first_chunk[o]
                            sp = c == last_chunk[o]
                            nc.tensor.matmul(
                                out=ps_n[o][:, :], lhsT=Oc[:, c, :],
                                rhs=XL[:, c, :], start=st, stop=sp)

                    # denominator matmuls AFTER the numerator group of the
                    # same PSUM bank has closed: interleaved accumulation
                    # groups within one bank corrupt each other (start
                    # resets the whole bank).
                    for i_t in range(len(ts)):
                        cs = [c for c in range(n) if owner[c] == i_t]
                        for k2, c in enumerate(cs):
                            nc.tensor.matmul(
                                out=ps_d[i_t][:, :], lhsT=Oc[:, c, :],
                                rhs=ones_c[:], start=k2 == 0,
                                stop=k2 == len(cs) - 1)

                    for i, t in enumerate(ts):
                        rec = sbp.tile([128, 1], F32, tag="rec")
                        nc.vector.reciprocal(rec[:], ps_d[i][:, :])
                        if l < T - 1:
                            h_sb = sbp.tile([128, D], BF16, tag="h")
                            nc.scalar.activation(
                                out=h_sb[:], in_=ps_n[i][:, :],
                                func=AF.Identity, scale=rec[:])
                            ps_tr = pst.tile([128, 128], BF16, space="PSUM",
                                             tag="tr")
                            nc.tensor.transpose(out=ps_tr[:], in_=h_sb[:],
                                                identity=ident[:])
                            nc.scalar.activation(
                                out=hT_t[t][:], in_=ps_tr[:], func=AF.Relu,
                                bias=bo_col[:], scale=1.0)
                        else:
                            o_sb = sbp.tile([128, D], F32, tag="o")
                            nc.scalar.activation(
                                out=o_sb[:], in_=ps_n[i][:, :],
                                func=AF.Identity, scale=rec[:])
                            nc.sync.dma_start(
                                out=out_t[t * 128 : (t + 1) * 128, :],
                                in_=o_sb[:])
                    pos += n

                    # pipeline: emit next layer's prologue for the tiles this
                    # supertile just produced; fire its AllGathers as soon as
                    # the corresponding xl_cc half is complete
                    if l < T - 1:
                        for t in ts:
                            prologue_tile(l + 1, t)
                        done_pro += len(ts)
                        if done_pro - len(ts) < H1 // 128 <= done_pro:
                            allgather(l + 1, 0)
                        if done_pro == TILES:
                            allgather(l + 1, 1)

    nc.compile()
    return nc


def _wrap_idx(idx_flat):
    """int16 idx vector -> [128, len/16] wrapped (16-partition) layout."""
    n = idx_flat.shape[0]
    assert n % 16 == 0
    w = idx_flat.reshape(n // 16, 16).T            # [16, n/16]
    return np.tile(w, (8, 1)).astype(np.int16)     # [128, n/16]


def _balance(deg):
    """Greedy bin-pack NP_ALL nodes into 392 buckets of exactly 128 slots,
    equalizing per-bucket edge counts. Returns (core_of, loc_of)."""
    import heapq

    nb = CORES * TILES
    order = np.argsort(-deg, kind="stable")
    heap = [(0, b) for b in range(nb)]
    heapq.heapify(heap)
    slots_used = np.zeros(nb, np.int32)
    core_of = np.empty(NP_ALL, np.int32)
    loc_of = np.empty(NP_ALL, np.int32)
    for n in order:
        e, b = heapq.heappop(heap)
        core_of[n] = b // TILES
        loc_of[n] = (b % TILES) * 128 + slots_used[b]
        slots_used[b] += 1
        if slots_used[b] < 128:
            heapq.heappush(heap, (e + int(deg[n]), b))
    return core_of, loc_of


def _prep(inputs):
    x = np.asarray(inputs["x"], np.float32)
    ei = np.asarray(inputs["edge_index"]).astype(np.int64)
    Wl0 = np.asarray(inputs["Wl0"], np.float32)
    Wr0 = np.asarray(inputs["Wr0"], np.float32)
    bl0 = np.asarray(inputs["bl0"], np.float32)
    br0 = np.asarray(inputs["br0"], np.float32)
    Wl = np.asarray(inputs["Wl"], np.float32)
    Wr = np.asarray(inputs["Wr"], np.float32)
    bl = np.asarray(inputs["bl"], np.float32)
    br = np.asarray(inputs["br"], np.float32)
    att = np.asarray(inputs["att"], np.float32)
    bias = np.asarray(inputs["bias"], np.float32)

    # --- attention folding: per layer, permute columns so a >= 0 comes
    # first (p_l columns) and scale columns by max(|a|, eps). ---
    perms, scales, p_cnt = [], [], []
    for l in range(T):
        a = att[l]
        pos = a >= 0
        perm = np.argsort(~pos, kind="stable")
        perms.append(perm)
        p_cnt.append(int(pos.sum()))
        scales.append(np.maximum(np.abs(a[perm]), 1e-12))

    # effective weights: fold perm+scale on output cols, and the inverse
    # of the previous layer's fold on the input rows
    Wl_eff, Wr_eff = [], []
    for l in range(T):
        wl_ = Wl0 if l == 0 else Wl[l - 1]
        wr_ = Wr0 if l == 0 else Wr[l - 1]
        if l > 0:
            unscale = (1.0 / scales[l - 1])[:, None]
            wl_ = wl_[perms[l - 1], :] * unscale
            wr_ = wr_[perms[l - 1], :] * unscale
        Wl_eff.append(wl_[:, perms[l]] * scales[l][None, :])
        Wr_eff.append(wr_[:, perms[l]] * scales[l][None, :])

    # xr bias (score side): (bl + br) folded
    brow2 = np.stack([
        ((bl0 + br0) if l == 0 else (bl[l - 1] + br[l - 1]))[perms[l]]
        * scales[l]
        for l in range(T)])
    # relu-step bias for inner layers: (bias_l + bl_l) folded
    bo_rows = []
    for l in range(T):
        b_ = bias[l] + (bl0 if l == 0 else bl[l - 1])
        bo_rows.append(b_[perms[l]] * scales[l])
    bout = np.stack(bo_rows).T.copy()   # [D, T]; col T-1 unused on-chip
    # host-side final unfold
    fin_perm = perms[T - 1]
    fin_scale = scales[T - 1]
    fin_bias = bias[T - 1] + bl[T - 2]

    # nodes 0..N-1 real, N..NP_ALL-1 virtual pads (degree-1 self loops keep
    # their softmax denominators finite)
    loop = np.arange(NP_ALL, dtype=np.int64)
    src = np.concatenate([ei[0], loop])
    dst = np.concatenate([ei[1], loop])

    deg = np.bincount(dst, minlength=NP_ALL)
    core_of, loc_of = _balance(deg)

    owner = core_of[dst].astype(np.int64)
    local = loc_of[dst].astype(np.int64)       # 0..SHP-1
    sc = core_of[src].astype(np.int64)
    sl = loc_of[src].astype(np.int64)
    s_half = (sl >= H1).astype(np.int64)
    g_row = np.where(s_half == 0, sc * H1 + sl, sc * H2 + (sl - H1))

    tile_of = local >> 7
    cnt = np.zeros((CORES, TILES, 2), np.int64)
    np.add.at(cnt, (owner, tile_of, s_half), 1)

    K1s = tuple(int(v) for v in
                np.ceil(cnt[:, :, 0].max(axis=0) / 128).astype(np.int64))
    K2s = tuple(int(v) for v in
                np.ceil(cnt[:, :, 1].max(axis=0) / 128).astype(np.int64))

    groups = [(t, t + 1) for t in range(0, TILES - 1, 2)] + [(TILES - 1,)]
    CH = sum(K1s) + sum(K2s)

    # per-core packing
    ixls, ohs, ohEs = [], [], []
    for c in range(CORES):
        sel = owner == c
        e_tile = tile_of[sel]
        e_half = s_half[sel]
        e_g = g_row[sel]
        e_dloc = local[sel]

        order = np.lexsort((e_dloc, e_half, e_tile))
        e_tile, e_half, e_g, e_dloc = (
            e_tile[order], e_half[order], e_g[order], e_dloc[order])
        bounds = np.searchsorted(
            e_tile * 2 + e_half, np.arange(TILES * 2 + 1))

        ixl = np.zeros(CH * 128, np.int64)
        oh = np.zeros((128, CH * 128), np.float32)
        ohE = np.zeros((128, CH * 128), np.float32)
        pos = 0
        for ts in groups:
            for half, Ks in ((0, K1s), (1, K2s)):
                for t in ts:
                    kk = Ks[t]
                    b0, b1 = bounds[t * 2 + half], bounds[t * 2 + half + 1]
                    ne = b1 - b0
                    assert ne <= kk * 128
                    sl2 = slice(pos * 128, pos * 128 + ne)
                    ixl[sl2] = e_g[b0:b1]
                    i_in = np.arange(ne)
                    dloc7 = e_dloc[b0:b1] & 127
                    oh[dloc7, (pos + (i_in >> 7)) * 128 + (i_in & 127)] = 1.0
                    ohE[i_in & 127, (pos + (i_in >> 7)) * 128 + dloc7] = 1.0
                    pos += kk
        assert pos == CH
        assert ixl.max() < 32768
        ixls.append(_wrap_idx(ixl.astype(np.int16)))
        ohs.append(oh)
        ohEs.append(ohE)

    def bf16(a):
        import jax.numpy as jnp
        return np.asarray(jnp.asarray(np.asarray(a, np.float32),
                                      dtype=jnp.bfloat16))

    Wlr0 = bf16(np.concatenate([Wl_eff[0], Wr_eff[0]], axis=1))
    Wlr_ = bf16(np.stack([
        np.concatenate([Wl_eff[l], Wr_eff[l]], axis=1)
        for l in range(1, T)]))
    common = dict(Wlr0=Wlr0, Wlr=Wlr_, br2=brow2.astype(np.float32),
                  bout=bout.astype(np.float32))
    in_maps = []
    nodes = np.arange(N)
    for c in range(CORES):
        xT_own = np.zeros((DIN, SHP), np.float32)
        m = core_of[:N] == c
        xT_own[:, loc_of[:N][m]] = x[nodes[m]].T
        in_maps.append(dict(common, xT_own=bf16(xT_own), ixl=ixls[c],
                            hs=bf16(ohs[c]), he=bf16(ohEs[c])))
    params = (K1s, K2s, tuple(p_cnt))
    fin = (fin_perm, fin_scale, fin_bias)
    return params, in_maps, (core_of, loc_of), fin


_CACHE = {}


def kernel(**inputs) -> np.ndarray:
    out, _ = _run(inputs)
    return out


def _run(inputs, **kw):
    params, in_maps, (core_of, loc_of), fin = _prep(inputs)
    if params not in _CACHE:
        _CACHE[params] = _build_nc(params)
    nc = _CACHE[params]
    res = run_bass_kernel_spmd(nc, in_maps, core_ids=list(range(CORES)), **kw)
    outs = np.stack([res.results[c]["out"] for c in range(CORES)])
    out = outs[core_of[:N], loc_of[:N]]
    # undo final-layer attention fold: unpermute + unscale cols, add bias
    fin_perm, fin_scale, fin_bias = fin
    res_f = np.empty_like(out)
    res_f[:, fin_perm] = out / fin_scale[None, :]
    res_f += fin_bias[None, :]
    return res_f.astype(np.float32), res


# revision 12
# speedup vs baseline: 1.9721x; 1.9721x over previous
"""GATv2 x5 (gnn_message_passing) on 8 Trainium2 NeuronCores.

Sharding: nodes partitioned across 8 cores by destination-node owner
(6250 nodes/core, padded to 6272 = 49 tiles of 128). Edges live with
their dst owner, grouped into 128-edge chunks per dst-tile, split by
src half (local row < 4096 vs >= 4096) so the two per-layer AllGathers
can overlap edge compute.

v2 changes vs the gather-everything baseline:
- xr[dst] rows are no longer DMA-gathered per edge. A static one-hot
  matrix OneHotS[s, e] (s = dst slot in tile, e = edge slot) is stored
  in DRAM (bf16) and loaded linearly per supertile; XR for a chunk is
  produced on the tensor engine as OneHotS^T @ xr_tile, accumulated in
  PSUM together with an identity-matmul of the gathered XL chunk, so
  S = XL + XR materializes directly in PSUM with no vector add.
- attention vector folded into the weights: Wl/Wr columns are permuted
  (a >= 0 first, p columns) and scaled by |a| (host side). The GATv2
  score becomes a plain column-sum of a per-sign Prelu: pos cols
  Prelu(alpha=.2), neg cols Prelu(alpha=5, scale=-.2). Scalar engine
  computes the activations (batched over 8-chunk sub-batches straight
  from PSUM), vector does one 3D tensor_reduce per sub-batch. The |a|
  scaling is undone by folding 1/|a| into the next layer's weight rows
  (final layer: on the host).
- the scaled one-hot Oc (softmax numerator scatter matrix) is built in
  two batched DVE ops per sub-batch using stride-0 broadcast APs
  (is_equal against iota, then multiply by exp(score)), replacing the
  per-chunk tensor_scalar ops that dominated DVE time.
All matmuls and gathers in bf16; accumulation, scores in fp32.
"""
import sys
import numpy as np

sys.path.insert(0, "/opt/trn_rl_repo")

import concourse.bass as bass
import concourse.bacc as bacc
import concourse.mybir as mybir
import concourse.tile as tile
from concourse.bass_utils import run_bass_kernel_spmd
from concourse.masks import make_identity

F32 = mybir.dt.float32
BF16 = mybir.dt.bfloat16
I16 = mybir.dt.int16
AF = mybir.ActivationFunctionType
OP = mybir.AluOpType
AX = mybir.AxisListType

N = 50000
DIN = 7
D = 128
T = 5
CORES = 8
SH = N // CORES            # 6250 nodes per core
TILES = 49
SHP = TILES * 128          # 6272 padded nodes per core
NP_ALL = CORES * SHP       # 50176 slots globally
H1 = 4096                  # local rows [0, 4096) -> AllGather half 1
H2 = SHP - H1              # 2176 rows           -> half 2 (8*H1 = 32768
                           # so half-1 gather rows exactly fit int16)
NEG = 0.2
G = 8                      # chunks per PSUM sub-batch


def _build_nc(params):
    """params = (K1s, K2s, ps): per-dst-tile chunk counts for src-half 1/2
    and per-layer positive-attention column counts."""
    K1s, K2s, ps = params
    CH = sum(K1s) + sum(K2s)   # total chunks per core

    # supertiles: pairs of dst tiles processed together
    groups = [(t, t + 1) for t in range(0, TILES - 1, 2)] + [(TILES - 1,)]
    self_qn = [0]   # round-robin SWDGE queue assignment

    nc = bacc.Bacc("TRN2", target_bir_lowering=False, debug=False,
                   num_devices=CORES, num_swdge_queues=4)

    xT_own = nc.dram_tensor("xT_own", [DIN, SHP], BF16, kind="ExternalInput")
    Wlr0 = nc.dram_tensor("Wlr0", [DIN, 2 * D], BF16, kind="ExternalInput")
    Wlr = nc.dram_tensor("Wlr", [T - 1, D, 2 * D], BF16, kind="ExternalInput")
    br2 = nc.dram_tensor("br2", [T, D], F32, kind="ExternalInput")
    bout = nc.dram_tensor("bout", [D, T], F32, kind="ExternalInput")
    ixl_i = nc.dram_tensor("ixl", [128, CH * 8], I16, kind="ExternalInput")
    hs_i = nc.dram_tensor("hs", [128, CH * 128], BF16, kind="ExternalInput")
    he_i = nc.dram_tensor("he", [128, CH * 128], BF16, kind="ExternalInput")

    out_t = nc.dram_tensor("out", [SHP, D], F32, kind="ExternalOutput")

    with tile.TileContext(nc) as tc:
        with (
            tc.tile_pool(name="pers", bufs=1) as pers,
            tc.tile_pool(name="wl", bufs=2) as wl,
            tc.tile_pool(name="edge", bufs=2) as ep,
            tc.tile_pool(name="sb", bufs=3) as sbp,
            tc.tile_pool(name="pro", bufs=1, space="PSUM") as psp,
            tc.tile_pool(name="spsum", bufs=2, space="PSUM") as psS,
            tc.tile_pool(name="aggn", bufs=2, space="PSUM") as psan,
            tc.tile_pool(name="tr", bufs=1, space="PSUM") as pst,
            tc.tile_pool(name="dram", bufs=2, space="DRAM") as dp,
        ):
            # --- persistent setup ---
            ident = pers.tile([128, 128], BF16)
            make_identity(nc, ident[:])
            ixl_sb = pers.tile([128, CH * 8], I16)
            nc.sync.dma_start(out=ixl_sb[:], in_=ixl_i[:, :])
            xT_sb = pers.tile([DIN, SHP], BF16)
            nc.sync.dma_start(out=xT_sb[:], in_=xT_own[:, :])
            ones_c = pers.tile([128, 1], BF16)
            nc.vector.memset(ones_c[:], 1.0)
            # per-tile hidden-state tiles: precise deps let next-layer
            # prologue (and its AllGather) overlap this layer's edge tail
            hT_t = [pers.tile([128, 128], BF16, name=f"hT{t}")
                    for t in range(TILES)]

            lw = {}    # layer -> weight/bias tiles
            ldr = {}   # layer -> dram staging tiles

            def load_weights(l):
                d = {}
                w_sb = wl.tile([128, 2 * D], BF16, tag="w", name=f"w{l}")
                if l == 0:
                    nc.sync.dma_start(out=w_sb[:DIN, :], in_=Wlr0[:, :])
                else:
                    nc.sync.dma_start(out=w_sb[:], in_=Wlr[l - 1, :, :])
                d["w"] = w_sb
                br2_b = wl.tile([128, D], F32, tag="br2", name=f"br2{l}")
                nc.sync.dma_start(
                    out=br2_b[:],
                    in_=br2[l : l + 1, :].partition_broadcast(128))
                d["br2"] = br2_b
                bo_col = wl.tile([128, 1], F32, tag="boc", name=f"boc{l}")
                nc.sync.dma_start(out=bo_col[:], in_=bout[:, l : l + 1])
                d["boc"] = bo_col
                lw[l] = d
                ldr[l] = dict(
                    xl_cc=dp.tile([SHP, D], BF16, tag="xlcc",
                                  name=f"xlcc{l}"),
                    xr_dr=dp.tile([SHP, D], BF16, tag="xrdr",
                                  name=f"xrdr{l}"),
                )

            def prologue_tile(l, m):
                d = lw[l]
                dr = ldr[l]
                ps2 = psp.tile([128, 2 * D], F32, space="PSUM", tag="pro",
                               name=f"pro{l}_{m}")
                if l == 0:
                    lhsT = xT_sb[:, m * 128 : (m + 1) * 128]
                    rhs = d["w"][:DIN, :]
                else:
                    lhsT = hT_t[m][:]
                    rhs = d["w"][:, :]
                nc.tensor.matmul(out=ps2[:], lhsT=lhsT, rhs=rhs,
                                 start=True, stop=True)
                xl_sb = sbp.tile([128, D], BF16, tag="xls")
                nc.scalar.activation(out=xl_sb[:], in_=ps2[:, :D],
                                     func=AF.Identity)
                nc.sync.dma_start(
                    out=dr["xl_cc"][m * 128 : (m + 1) * 128, :], in_=xl_sb[:])
                xr_sb = sbp.tile([128, D], BF16, tag="xrs")
                nc.vector.tensor_tensor(
                    out=xr_sb[:], in0=ps2[:, D:], in1=d["br2"][:], op=OP.add)
                nc.sync.dma_start(
                    out=dr["xr_dr"][m * 128 : (m + 1) * 128, :], in_=xr_sb[:])

            def allgather(l, half):
                dr = ldr[l]
                if half == 0:
                    xl_h = dp.tile([CORES * H1, D], BF16, tag="xlh1",
                                   name=f"xlh1_{l}")
                    src = dr["xl_cc"][0:H1, :]
                    dr["h1"] = xl_h
                else:
                    xl_h = dp.tile([CORES * H2, D], BF16, tag="xlh2",
                                   name=f"xlh2_{l}")
                    src = dr["xl_cc"][H1:SHP, :]
                    dr["h2"] = xl_h
                nc.gpsimd.collective_compute(
                    "AllGather", OP.bypass,
                    replica_groups=[list(range(CORES))],
                    ins=[src.opt()],
                    outs=[xl_h[:, :].opt()],
                )

            # layer-0 prologue + gathers up front
            load_weights(0)
            for m in range(TILES):
                prologue_tile(0, m)
            allgather(0, 0)
            allgather(0, 1)

            for l in range(T):
                d_l = lw[l]
                bo_col = d_l["boc"]
                p_l = ps[l]
                xl_h1 = ldr[l]["h1"]
                xl_h2 = ldr[l]["h2"]
                xr_dr = ldr[l]["xr_dr"]
                if l < T - 1:
                    load_weights(l + 1)
                done_pro = 0

                # --- edge stage: per supertile (pair of dst tiles) ---
                pos = 0
                for ts in groups:
                    k1s = [K1s[t] for t in ts]
                    k2s = [K2s[t] for t in ts]
                    nA = sum(k1s)
                    nB = sum(k2s)
                    n = nA + nB
                    # chunk position -> (member index, dst-tile)
                    # layout: [A of ts[0] | A of ts[1] | B of ts[0] | B of ts[1]]
                    owner = []
                    for i, t in enumerate(ts):
                        owner += [i] * k1s[i]
                    for i, t in enumerate(ts):
                        owner += [i] * k2s[i]
                    first_chunk = {}
                    last_chunk = {}
                    for c, o in enumerate(owner):
                        if o not in first_chunk:
                            first_chunk[o] = c
                        last_chunk[o] = c

                    # dma_gather is capped at 1024 idxs (8 chunks of 128) by
                    # the SWDGE descriptor ring; split into sub-gathers spread
                    # over the 4 SWDGE queues.
                    def gathers(dst, src_ap, idx_sb, c0, nch):
                        off = 0
                        while off < nch:
                            g = min(8, nch - off)
                            nc.gpsimd.dma_gather(
                                dst[:, off : off + g, :], src_ap,
                                idx_sb[:, (c0 + off) * 8 : (c0 + off + g) * 8],
                                g * 128, g * 128, D,
                                queue_num=self_qn[0] % 4)
                            self_qn[0] += 1
                            off += g

                    XL = ep.tile([128, n, D], BF16, tag="XL", bufs=3)
                    if nA:
                        gathers(XL[:, :, :], xl_h1[:, :], ixl_sb, pos, nA)
                    if nB:
                        gathers(XL[:, nA:, :], xl_h2[:, :], ixl_sb,
                                pos + nA, nB)

                    # static one-hots (linear DMA) + xr rows of the 1-2 tiles
                    Hs = ep.tile([128, n, 128], BF16, tag="Hs", bufs=3)
                    nc.sync.dma_start(
                        out=Hs[:, :, :],
                        in_=hs_i[:, pos * 128 : (pos + n) * 128])
                    He = ep.tile([128, n, 128], BF16, tag="He", bufs=3)
                    nc.sync.dma_start(
                        out=He[:, :, :],
                        in_=he_i[:, pos * 128 : (pos + n) * 128])
                    xr_st = []
                    for i, t in enumerate(ts):
                        xr_i = sbp.tile([128, D], BF16, tag=f"xrst{i}")
                        nc.sync.dma_start(
                            out=xr_i[:],
                            in_=xr_dr[t * 128 : (t + 1) * 128, :])
                        xr_st.append(xr_i)

                    e_t = sbp.tile([128, n], F32, tag="e")
                    ex_t = sbp.tile([128, n], F32, tag="ex")
                    Oc = ep.tile([128, n, 128], BF16, tag="Oc")

                    ps_n = []
                    ps_d = []
                    for i in range(len(ts)):
                        pnd = psan.tile([128, D + 1], F32, space="PSUM",
                                        tag="aggn", name=f"psnd{i}")
                        ps_n.append(pnd[:, :D])
                        ps_d.append(pnd[:, D : D + 1])

                    for b0 in range(0, n, G):
                        g = min(G, n - b0)
                        S_ps = psS.tile([128, G, 128], F32, space="PSUM",
                                        tag="sps")
                        for j in range(g):
                            c = b0 + j
                            nc.tensor.matmul(
                                out=S_ps[:, j, :], lhsT=Hs[:, c, :],
                                rhs=xr_st[owner[c]][:],
                                start=True, stop=False)
                            nc.tensor.matmul(
                                out=S_ps[:, j, :], lhsT=ident[:],
                                rhs=XL[:, c, :], start=False, stop=True)
                        # score: sign-folded prelu + column sum
                        Lpp = sbp.tile([128, G, 128], BF16, tag="lpp")
                        if p_l > 0:
                            nc.scalar.activation(
                                out=Lpp[:, :g, :p_l], in_=S_ps[:, :g, :p_l],
                                func=AF.Prelu, alpha=NEG)
                        if p_l < D:
                            nc.scalar.activation(
                                out=Lpp[:, :g, p_l:], in_=S_ps[:, :g, p_l:],
                                func=AF.Prelu, alpha=1.0 / NEG, scale=-NEG)
                        nc.vector.tensor_reduce(
                            out=e_t[:, b0 : b0 + g], in_=Lpp[:, :g, :],
                            axis=AX.X, op=OP.add)
                        nc.scalar.activation(
                            out=ex_t[:, b0 : b0 + g], in_=e_t[:, b0 : b0 + g],
                            func=AF.Exp)
                        # scaled one-hot Oc = OneHotE * ex (one-hot from DRAM)
                        nc.vector.tensor_tensor(
                            out=Oc[:, b0 : b0 + g, :],
                            in0=He[:, b0 : b0 + g, :],
                            in1=ex_t[:, b0 : b0 + g].unsqueeze(2)
                                .broadcast_to([128, g, 128]),
                            op=OP.mult)
                        for j in range(g):
                            c = b0 + j
                            o = owner[c]
                            st = c == first_chunk[o]
                            sp = c == last_chunk[o]
                            nc.tensor.matmul(
                                out=ps_n[o][:, :], lhsT=Oc[:, c, :],
                                rhs=XL[:, c, :], start=st, stop=sp)

                    # denominator matmuls AFTER the numerator group of the
                    # same PSUM bank has closed: interleaved accumulation
                    # groups within one bank corrupt each other (start
                    # resets the whole bank).
                    for i_t in range(len(ts)):
                        cs = [c for c in range(n) if owner[c] == i_t]
                        for k2, c in enumerate(cs):
                            nc.tensor.matmul(
                                out=ps_d[i_t][:, :], lhsT=Oc[:, c, :],
                                rhs=ones_c[:], start=k2 == 0,
                                stop=k2 == len(cs) - 1)

                    for i, t in enumerate(ts):
                        rec = sbp.tile([128, 1], F32, tag="rec")
                        nc.vector.reciprocal(rec[:], ps_d[i][:, :])
                        if l < T - 1:
                            h_sb = sbp.tile([128, D], BF16, tag="h")
                            nc.scalar.activation(
                                out=h_sb[:], in_=ps_n[i][:, :],
                                func=AF.Identity, scale=rec[:])
                            ps_tr = pst.tile([128, 128], BF16, space="PSUM",
                                             tag="tr")
                            nc.tensor.transpose(out=ps_tr[:], in_=h_sb[:],
                                                identity=ident[:])
                            nc.scalar.activation(
                                out=hT_t[t][:], in_=ps_tr[:], func=AF.Relu,
                                bias=bo_col[:], scale=1.0)
                        else:
                            o_sb = sbp.tile([128, D], F32, tag="o")
                            nc.scalar.activation(
                                out=o_sb[:], in_=ps_n[i][:, :],
                                func=AF.Identity, scale=rec[:])
                            nc.sync.dma_start(
                                out=out_t[t * 128 : (t + 1) * 128, :],
                                in_=o_sb[:])
                    pos += n

                    # pipeline: emit next layer's prologue for the tiles this
                    # supertile just produced; fire its AllGathers as soon as
                    # the corresponding xl_cc half is complete
                    if l < T - 1:
                        for t in ts:
                            prologue_tile(l + 1, t)
                        done_pro += len(ts)
                        if done_pro - len(ts) < H1 // 128 <= done_pro:
                            allgather(l + 1, 0)
                        if done_pro == TILES:
                            allgather(l + 1, 1)

    nc.compile()
    return nc


def _wrap_idx(idx_flat):
    """int16 idx vector -> [128, len/16] wrapped (16-partition) layout."""
    n = idx_flat.shape[0]
    assert n % 16 == 0
    w = idx_flat.reshape(n // 16, 16).T            # [16, n/16]
    return np.tile(w, (8, 1)).astype(np.int16)     # [128, n/16]


def _balance(deg):
    """Greedy bin-pack NP_ALL nodes into 392 buckets of exactly 128 slots,
    equalizing per-bucket edge counts. Returns (core_of, loc_of)."""
    import heapq

    nb = CORES * TILES
    order = np.argsort(-deg, kind="stable")
    heap = [(0, b) for b in range(nb)]
    heapq.heapify(heap)
    slots_used = np.zeros(nb, np.int32)
    core_of = np.empty(NP_ALL, np.int32)
    loc_of = np.empty(NP_ALL, np.int32)
    for n in order:
        e, b = heapq.heappop(heap)
        core_of[n] = b // TILES
        loc_of[n] = (b % TILES) * 128 + slots_used[b]
        slots_used[b] += 1
        if slots_used[b] < 128:
            heapq.heappush(heap, (e + int(deg[n]), b))
    return core_of, loc_of


def _prep(inputs):
    x = np.asarray(inputs["x"], np.float32)
    ei = np.asarray(inputs["edge_index"]).astype(np.int64)
    Wl0 = np.asarray(inputs["Wl0"], np.float32)
    Wr0 = np.asarray(inputs["Wr0"], np.float32)
    bl0 = np.asarray(inputs["bl0"], np.float32)
    br0 = np.asarray(inputs["br0"], np.float32)
    Wl = np.asarray(inputs["Wl"], np.float32)
    Wr = np.asarray(inputs["Wr"], np.float32)
    bl = np.asarray(inputs["bl"], np.float32)
    br = np.asarray(inputs["br"], np.float32)
    att = np.asarray(inputs["att"], np.float32)
    bias = np.asarray(inputs["bias"], np.float32)

    # --- attention folding: per layer, permute columns so a >= 0 comes
    # first (p_l columns) and scale columns by max(|a|, eps). ---
    perms, scales, p_cnt = [], [], []
    for l in range(T):
        a = att[l]
        pos = a >= 0
        perm = np.argsort(~pos, kind="stable")
        perms.append(perm)
        p_cnt.append(int(pos.sum()))
        scales.append(np.maximum(np.abs(a[perm]), 1e-12))

    # effective weights: fold perm+scale on output cols, and the inverse
    # of the previous layer's fold on the input rows
    Wl_eff, Wr_eff = [], []
    for l in range(T):
        wl_ = Wl0 if l == 0 else Wl[l - 1]
        wr_ = Wr0 if l == 0 else Wr[l - 1]
        if l > 0:
            unscale = (1.0 / scales[l - 1])[:, None]
            wl_ = wl_[perms[l - 1], :] * unscale
            wr_ = wr_[perms[l - 1], :] * unscale
        Wl_eff.append(wl_[:, perms[l]] * scales[l][None, :])
        Wr_eff.append(wr_[:, perms[l]] * scales[l][None, :])

    # xr bias (score side): (bl + br) folded
    brow2 = np.stack([
        ((bl0 + br0) if l == 0 else (bl[l - 1] + br[l - 1]))[perms[l]]
        * scales[l]
        for l in range(T)])
    # relu-step bias for inner layers: (bias_l + bl_l) folded
    bo_rows = []
    for l in range(T):
        b_ = bias[l] + (bl0 if l == 0 else bl[l - 1])
        bo_rows.append(b_[perms[l]] * scales[l])
    bout = np.stack(bo_rows).T.copy()   # [D, T]; col T-1 unused on-chip
    # host-side final unfold
    fin_perm = perms[T - 1]
    fin_scale = scales[T - 1]
    fin_bias = bias[T - 1] + bl[T - 2]

    # nodes 0..N-1 real, N..NP_ALL-1 virtual pads (degree-1 self loops keep
    # their softmax denominators finite)
    loop = np.arange(NP_ALL, dtype=np.int64)
    src = np.concatenate([ei[0], loop])
    dst = np.concatenate([ei[1], loop])

    deg = np.bincount(dst, minlength=NP_ALL)
    core_of, loc_of = _balance(deg)

    owner = core_of[dst].astype(np.int64)
    local = loc_of[dst].astype(np.int64)       # 0..SHP-1
    sc = core_of[src].astype(np.int64)
    sl = loc_of[src].astype(np.int64)
    s_half = (sl >= H1).astype(np.int64)
    g_row = np.where(s_half == 0, sc * H1 + sl, sc * H2 + (sl - H1))

    tile_of = local >> 7
    cnt = np.zeros((CORES, TILES, 2), np.int64)
    np.add.at(cnt, (owner, tile_of, s_half), 1)

    K1s = tuple(int(v) for v in
                np.ceil(cnt[:, :, 0].max(axis=0) / 128).astype(np.int64))
    K2s = tuple(int(v) for v in
                np.ceil(cnt[:, :, 1].max(axis=0) / 128).astype(np.int64))

    groups = [(t, t + 1) for t in range(0, TILES - 1, 2)] + [(TILES - 1,)]
    CH = sum(K1s) + sum(K2s)

    # per-core packing
    ixls, ohs, ohEs = [], [], []
    for c in range(CORES):
        sel = owner == c
        e_tile = tile_of[sel]
        e_half = s_half[sel]
        e_g = g_row[sel]
        e_dloc = local[sel]

        order = np.lexsort((e_dloc, e_half, e_tile))
        e_tile, e_half, e_g, e_dloc = (
            e_tile[order], e_half[order], e_g[order], e_dloc[order])
        bounds = np.searchsorted(
            e_tile * 2 + e_half, np.arange(TILES * 2 + 1))

        ixl = np.zeros(CH * 128, np.int64)
        oh = np.zeros((128, CH * 128), np.float32)
        ohE = np.zeros((128, CH * 128), np.float32)
        pos = 0
        for ts in groups:
            for half, Ks in ((0, K1s), (1, K2s)):
                for t in ts:
                    kk = Ks[t]
                    b0, b1 = bounds[t * 2 + half], bounds[t * 2 + half + 1]
                    ne = b1 - b0
                    assert ne <= kk * 128
                    sl2 = slice(pos * 128, pos * 128 + ne)
                    ixl[sl2] = e_g[b0:b1]
                    i_in = np.arange(ne)
                    dloc7 = e_dloc[b0:b1] & 127
                    oh[dloc7, (pos + (i_in >> 7)) * 128 + (i_in & 127)] = 1.0
                    ohE[i_in & 127, (pos + (i_in >> 7)) * 128 + dloc7] = 1.0
                    pos += kk
        assert pos == CH
        assert ixl.max() < 32768
        ixls.append(_wrap_idx(ixl.astype(np.int16)))
        ohs.append(oh)
        ohEs.append(ohE)

    def bf16(a):
        import jax.numpy as jnp
        return np.asarray(jnp.asarray(np.asarray(a, np.float32),
                                      dtype=jnp.bfloat16))

    Wlr0 = bf16(np.concatenate([Wl_eff[0], Wr_eff[0]], axis=1))
    Wlr_ = bf16(np.stack([
        np.concatenate([Wl_eff[l], Wr_eff[l]], axis=1)
        for l in range(1, T)]))
    common = dict(Wlr0=Wlr0, Wlr=Wlr_, br2=brow2.astype(np.float32),
                  bout=bout.astype(np.float32))
    in_maps = []
    nodes = np.arange(N)
    for c in range(CORES):
        xT_own = np.zeros((DIN, SHP), np.float32)
        m = core_of[:N] == c
        xT_own[:, loc_of[:N][m]] = x[nodes[m]].T
        in_maps.append(dict(common, xT_own=bf16(xT_own), ixl=ixls[c],
                            hs=bf16(ohs[c]), he=bf16(ohEs[c])))
    params = (K1s, K2s, tuple(p_cnt))
    fin = (fin_perm, fin_scale, fin_bias)
    return params, in_maps, (core_of, loc_of), fin


_CACHE = {}


def kernel(**inputs) -> np.ndarray:
    out, _ = _run(inputs)
    return out


def _run(inputs, **kw):
    params, in_maps, (core_of, loc_of), fin = _prep(inputs)
    if params not in _CACHE:
        _CACHE[params] = _build_nc(params)
    nc = _CACHE[params]
    res = run_bass_kernel_spmd(nc, in_maps, core_ids=list(range(CORES)), **kw)
    outs = np.stack([res.results[c]["out"] for c in range(CORES)])
    out = outs[core_of[:N], loc_of[:N]]
    # undo final-layer attention fold: unpermute + unscale cols, add bias
    fin_perm, fin_scale, fin_bias = fin
    res_f = np.empty_like(out)
    res_f[:, fin_perm] = out / fin_scale[None, :]
    res_f += fin_bias[None, :]
    return res_f.astype(np.float32), res


# revision 14
# speedup vs baseline: 21.0032x; 10.6504x over previous
"""GATv2 x5 (gnn_message_passing) on 8 Trainium2 NeuronCores.

Sharding: nodes partitioned across 8 cores by destination-node owner
(6250 nodes/core, padded to 6272 = 49 tiles of 128). Edges live with
their dst owner, grouped into 128-edge chunks per dst-tile, split by
src half (local row < 4096 vs >= 4096) so the two per-layer AllGathers
can overlap edge compute.

v2 changes vs the gather-everything baseline:
- xr[dst] rows are no longer DMA-gathered per edge. A static one-hot
  matrix OneHotS[s, e] (s = dst slot in tile, e = edge slot) is stored
  in DRAM (bf16) and loaded linearly per supertile; XR for a chunk is
  produced on the tensor engine as OneHotS^T @ xr_tile, accumulated in
  PSUM together with an identity-matmul of the gathered XL chunk, so
  S = XL + XR materializes directly in PSUM with no vector add.
- attention vector folded into the weights: Wl/Wr columns are permuted
  (a >= 0 first, p columns) and scaled by |a| (host side). The GATv2
  score becomes a plain column-sum of a per-sign Prelu: pos cols
  Prelu(alpha=.2), neg cols Prelu(alpha=5, scale=-.2). Scalar engine
  computes the activations (batched over 8-chunk sub-batches straight
  from PSUM), vector does one 3D tensor_reduce per sub-batch. The |a|
  scaling is undone by folding 1/|a| into the next layer's weight rows
  (final layer: on the host).
- the scaled one-hot Oc (softmax numerator scatter matrix) is built in
  two batched DVE ops per sub-batch using stride-0 broadcast APs
  (is_equal against iota, then multiply by exp(score)), replacing the
  per-chunk tensor_scalar ops that dominated DVE time.
All matmuls and gathers in bf16; accumulation, scores in fp32.
"""
import sys
import numpy as np

sys.path.insert(0, "/opt/trn_rl_repo")

import concourse.bass as bass
import concourse.bacc as bacc
import concourse.mybir as mybir
import concourse.tile as tile
from concourse.bass_utils import run_bass_kernel_spmd
from concourse.masks import make_identity

F32 = mybir.dt.float32
BF16 = mybir.dt.bfloat16
I16 = mybir.dt.int16
AF = mybir.ActivationFunctionType
OP = mybir.AluOpType
AX = mybir.AxisListType

N = 50000
DIN = 7
D = 128
T = 5
CORES = 8
SH = N // CORES            # 6250 nodes per core
TILES = 49
SHP = TILES * 128          # 6272 padded nodes per core
NP_ALL = CORES * SHP       # 50176 slots globally
H1 = 4096                  # local rows [0, 4096) -> AllGather half 1
H2 = SHP - H1              # 2176 rows           -> half 2 (8*H1 = 32768
                           # so half-1 gather rows exactly fit int16)
NEG = 0.2
G = 8                      # chunks per PSUM sub-batch


def _build_nc(params):
    """params = (K1s, K2s, ps): per-dst-tile chunk counts for src-half 1/2
    and per-layer positive-attention column counts."""
    K1s, K2s, ps = params
    CH = sum(K1s) + sum(K2s)   # total chunks per core

    # supertiles: pairs of dst tiles processed together
    groups = [(t, t + 1) for t in range(0, TILES - 1, 2)] + [(TILES - 1,)]
    self_qn = [0]   # round-robin SWDGE queue assignment

    nc = bacc.Bacc("TRN2", target_bir_lowering=False, debug=False,
                   num_devices=CORES, num_swdge_queues=4)

    xT_own = nc.dram_tensor("xT_own", [DIN, SHP], BF16, kind="ExternalInput")
    Wlr0 = nc.dram_tensor("Wlr0", [DIN, 2 * D], BF16, kind="ExternalInput")
    Wlr = nc.dram_tensor("Wlr", [T - 1, D, 2 * D], BF16, kind="ExternalInput")
    br2 = nc.dram_tensor("br2", [T, D], F32, kind="ExternalInput")
    bout = nc.dram_tensor("bout", [D, T], F32, kind="ExternalInput")
    ixl_i = nc.dram_tensor("ixl", [128, CH * 8], I16, kind="ExternalInput")
    hs_i = nc.dram_tensor("hs", [128, CH * 128], BF16, kind="ExternalInput")
    he_i = nc.dram_tensor("he", [128, CH * 128], BF16, kind="ExternalInput")

    out_t = nc.dram_tensor("out", [SHP, D], F32, kind="ExternalOutput")

    with tile.TileContext(nc) as tc:
        with (
            tc.tile_pool(name="pers", bufs=1) as pers,
            tc.tile_pool(name="wl", bufs=2) as wl,
            tc.tile_pool(name="edge", bufs=2) as ep,
            tc.tile_pool(name="sb", bufs=3) as sbp,
            tc.tile_pool(name="pro", bufs=1, space="PSUM") as psp,
            tc.tile_pool(name="spsum", bufs=2, space="PSUM") as psS,
            tc.tile_pool(name="aggn", bufs=2, space="PSUM") as psan,
            tc.tile_pool(name="tr", bufs=1, space="PSUM") as pst,
            tc.tile_pool(name="dram", bufs=2, space="DRAM") as dp,
        ):
            # --- persistent setup ---
            ident = pers.tile([128, 128], BF16)
            make_identity(nc, ident[:])
            ixl_sb = pers.tile([128, CH * 8], I16)
            nc.sync.dma_start(out=ixl_sb[:], in_=ixl_i[:, :])
            xT_sb = pers.tile([DIN, SHP], BF16)
            nc.sync.dma_start(out=xT_sb[:], in_=xT_own[:, :])
            ones_c = pers.tile([128, 1], BF16)
            nc.vector.memset(ones_c[:], 1.0)
            # per-tile hidden-state tiles: precise deps let next-layer
            # prologue (and its AllGather) overlap this layer's edge tail
            hT_t = [pers.tile([128, 128], BF16, name=f"hT{t}")
                    for t in range(TILES)]

            lw = {}    # layer -> weight/bias tiles
            ldr = {}   # layer -> dram staging tiles

            def load_weights(l):
                d = {}
                w_sb = wl.tile([128, 2 * D], BF16, tag="w", name=f"w{l}")
                if l == 0:
                    nc.sync.dma_start(out=w_sb[:DIN, :], in_=Wlr0[:, :])
                else:
                    nc.sync.dma_start(out=w_sb[:], in_=Wlr[l - 1, :, :])
                d["w"] = w_sb
                br2_b = wl.tile([128, D], F32, tag="br2", name=f"br2{l}")
                nc.sync.dma_start(
                    out=br2_b[:],
                    in_=br2[l : l + 1, :].partition_broadcast(128))
                d["br2"] = br2_b
                bo_col = wl.tile([128, 1], F32, tag="boc", name=f"boc{l}")
                nc.sync.dma_start(out=bo_col[:], in_=bout[:, l : l + 1])
                d["boc"] = bo_col
                lw[l] = d
                ldr[l] = dict(
                    xl_cc=dp.tile([SHP, D], BF16, tag="xlcc",
                                  name=f"xlcc{l}"),
                    xr_dr=dp.tile([SHP, D], BF16, tag="xrdr",
                                  name=f"xrdr{l}"),
                )

            def prologue_tile(l, m):
                d = lw[l]
                dr = ldr[l]
                ps2 = psp.tile([128, 2 * D], F32, space="PSUM", tag="pro",
                               name=f"pro{l}_{m}")
                if l == 0:
                    lhsT = xT_sb[:, m * 128 : (m + 1) * 128]
                    rhs = d["w"][:DIN, :]
                else:
                    lhsT = hT_t[m][:]
                    rhs = d["w"][:, :]
                nc.tensor.matmul(out=ps2[:], lhsT=lhsT, rhs=rhs,
                                 start=True, stop=True)
                xl_sb = sbp.tile([128, D], BF16, tag="xls")
                nc.scalar.activation(out=xl_sb[:], in_=ps2[:, :D],
                                     func=AF.Identity)
                nc.sync.dma_start(
                    out=dr["xl_cc"][m * 128 : (m + 1) * 128, :], in_=xl_sb[:])
                xr_sb = sbp.tile([128, D], BF16, tag="xrs")
                nc.vector.tensor_tensor(
                    out=xr_sb[:], in0=ps2[:, D:], in1=d["br2"][:], op=OP.add)
                nc.sync.dma_start(
                    out=dr["xr_dr"][m * 128 : (m + 1) * 128, :], in_=xr_sb[:])

            def allgather(l, half):
                dr = ldr[l]
                if half == 0:
                    xl_h = dp.tile([CORES * H1, D], BF16, tag="xlh1",
                                   name=f"xlh1_{l}")
                    src = dr["xl_cc"][0:H1, :]
                    dr["h1"] = xl_h
                else:
                    xl_h = dp.tile([CORES * H2, D], BF16, tag="xlh2",
                                   name=f"xlh2_{l}")
                    src = dr["xl_cc"][H1:SHP, :]
                    dr["h2"] = xl_h
                nc.gpsimd.collective_compute(
                    "AllGather", OP.bypass,
                    replica_groups=[list(range(CORES))],
                    ins=[src.opt()],
                    outs=[xl_h[:, :].opt()],
                )

            # layer-0 prologue + gathers up front
            load_weights(0)
            for m in range(TILES):
                prologue_tile(0, m)
            allgather(0, 0)
            allgather(0, 1)

            for l in range(T):
                d_l = lw[l]
                bo_col = d_l["boc"]
                p_l = ps[l]
                xl_h1 = ldr[l]["h1"]
                xl_h2 = ldr[l]["h2"]
                xr_dr = ldr[l]["xr_dr"]
                if l < T - 1:
                    load_weights(l + 1)
                done_pro = 0

                # --- edge stage: per supertile (pair of dst tiles) ---
                pos = 0
                for ts in groups:
                    k1s = [K1s[t] for t in ts]
                    k2s = [K2s[t] for t in ts]
                    nA = sum(k1s)
                    nB = sum(k2s)
                    n = nA + nB
                    # chunk position -> (member index, dst-tile)
                    # layout: [A of ts[0] | A of ts[1] | B of ts[0] | B of ts[1]]
                    owner = []
                    for i, t in enumerate(ts):
                        owner += [i] * k1s[i]
                    for i, t in enumerate(ts):
                        owner += [i] * k2s[i]
                    first_chunk = {}
                    last_chunk = {}
                    for c, o in enumerate(owner):
                        if o not in first_chunk:
                            first_chunk[o] = c
                        last_chunk[o] = c

                    # dma_gather is capped at 1024 idxs (8 chunks of 128) by
                    # the SWDGE descriptor ring; split into sub-gathers spread
                    # over the 4 SWDGE queues.
                    def gathers(dst, src_ap, idx_sb, c0, nch):
                        off = 0
                        while off < nch:
                            g = min(8, nch - off)
                            nc.gpsimd.dma_gather(
                                dst[:, off : off + g, :], src_ap,
                                idx_sb[:, (c0 + off) * 8 : (c0 + off + g) * 8],
                                g * 128, g * 128, D,
                                queue_num=self_qn[0] % 4)
                            self_qn[0] += 1
                            off += g

                    XL = ep.tile([128, n, D], BF16, tag="XL", bufs=4)
                    if nA:
                        gathers(XL[:, :, :], xl_h1[:, :], ixl_sb, pos, nA)
                    if nB:
                        gathers(XL[:, nA:, :], xl_h2[:, :], ixl_sb,
                                pos + nA, nB)

                    # static one-hots (linear DMA) + xr rows of the 1-2 tiles
                    Hs = ep.tile([128, n, 128], BF16, tag="Hs", bufs=4)
                    nc.sync.dma_start(
                        out=Hs[:, :, :],
                        in_=hs_i[:, pos * 128 : (pos + n) * 128])
                    He = ep.tile([128, n, 128], BF16, tag="He", bufs=4)
                    nc.sync.dma_start(
                        out=He[:, :, :],
                        in_=he_i[:, pos * 128 : (pos + n) * 128])
                    xr_st = []
                    for i, t in enumerate(ts):
                        xr_i = sbp.tile([128, D], BF16, tag=f"xrst{i}")
                        nc.sync.dma_start(
                            out=xr_i[:],
                            in_=xr_dr[t * 128 : (t + 1) * 128, :])
                        xr_st.append(xr_i)

                    e_t = sbp.tile([128, n], F32, tag="e")
                    ex_t = sbp.tile([128, n], F32, tag="ex")
                    Oc = ep.tile([128, n, 128], BF16, tag="Oc")

                    ps_n = []
                    ps_d = []
                    for i in range(len(ts)):
                        pnd = psan.tile([128, D + 1], F32, space="PSUM",
                                        tag="aggn", name=f"psnd{i}")
                        ps_n.append(pnd[:, :D])
                        ps_d.append(pnd[:, D : D + 1])

                    for b0 in range(0, n, G):
                        g = min(G, n - b0)
                        S_ps = psS.tile([128, G, 128], F32, space="PSUM",
                                        tag="sps")
                        for j in range(g):
                            c = b0 + j
                            nc.tensor.matmul(
                                out=S_ps[:, j, :], lhsT=Hs[:, c, :],
                                rhs=xr_st[owner[c]][:],
                                start=True, stop=False)
                            nc.tensor.matmul(
                                out=S_ps[:, j, :], lhsT=ident[:],
                                rhs=XL[:, c, :], start=False, stop=True)
                        # score: sign-folded prelu + column sum
                        Lpp = sbp.tile([128, G, 128], BF16, tag="lpp")
                        if p_l > 0:
                            nc.scalar.activation(
                                out=Lpp[:, :g, :p_l], in_=S_ps[:, :g, :p_l],
                                func=AF.Prelu, alpha=NEG)
                        if p_l < D:
                            nc.scalar.activation(
                                out=Lpp[:, :g, p_l:], in_=S_ps[:, :g, p_l:],
                                func=AF.Prelu, alpha=1.0 / NEG, scale=-NEG)
                        nc.vector.tensor_reduce(
                            out=e_t[:, b0 : b0 + g], in_=Lpp[:, :g, :],
                            axis=AX.X, op=OP.add)
                        nc.scalar.activation(
                            out=ex_t[:, b0 : b0 + g], in_=e_t[:, b0 : b0 + g],
                            func=AF.Exp)
                        # scaled one-hot Oc = OneHotE * ex (one-hot from DRAM)
                        nc.vector.tensor_tensor(
                            out=Oc[:, b0 : b0 + g, :],
                            in0=He[:, b0 : b0 + g, :],
                            in1=ex_t[:, b0 : b0 + g].unsqueeze(2)
                                .broadcast_to([128, g, 128]),
                            op=OP.mult)
                        for j in range(g):
                            c = b0 + j
                            o = owner[c]
                            st = c == first_chunk[o]
                            sp = c == last_chunk[o]
                            nc.tensor.matmul(
                                out=ps_n[o][:, :], lhsT=Oc[:, c, :],
                                rhs=XL[:, c, :], start=st, stop=sp)

                    # denominator matmuls AFTER the numerator group of the
                    # same PSUM bank has closed: interleaved accumulation
                    # groups within one bank corrupt each other (start
                    # resets the whole bank).
                    for i_t in range(len(ts)):
                        cs = [c for c in range(n) if owner[c] == i_t]
                        for k2, c in enumerate(cs):
                            nc.tensor.matmul(
                                out=ps_d[i_t][:, :], lhsT=Oc[:, c, :],
                                rhs=ones_c[:], start=k2 == 0,
                                stop=k2 == len(cs) - 1)

                    for i, t in enumerate(ts):
                        rec = sbp.tile([128, 1], F32, tag="rec")
                        nc.vector.reciprocal(rec[:], ps_d[i][:, :])
                        if l < T - 1:
                            h_sb = sbp.tile([128, D], BF16, tag="h")
                            nc.scalar.activation(
                                out=h_sb[:], in_=ps_n[i][:, :],
                                func=AF.Identity, scale=rec[:])
                            ps_tr = pst.tile([128, 128], BF16, space="PSUM",
                                             tag="tr")
                            nc.tensor.transpose(out=ps_tr[:], in_=h_sb[:],
                                                identity=ident[:])
                            nc.scalar.activation(
                                out=hT_t[t][:], in_=ps_tr[:], func=AF.Relu,
                                bias=bo_col[:], scale=1.0)
                        else:
                            o_sb = sbp.tile([128, D], F32, tag="o")
                            nc.scalar.activation(
                                out=o_sb[:], in_=ps_n[i][:, :],
                                func=AF.Identity, scale=rec[:])
                            nc.sync.dma_start(
                                out=out_t[t * 128 : (t + 1) * 128, :],
                                in_=o_sb[:])
                    pos += n

                    # pipeline: emit next layer's prologue for the tiles this
                    # supertile just produced; fire its AllGathers as soon as
                    # the corresponding xl_cc half is complete
                    if l < T - 1:
                        for t in ts:
                            prologue_tile(l + 1, t)
                        done_pro += len(ts)
                        if done_pro - len(ts) < H1 // 128 <= done_pro:
                            allgather(l + 1, 0)
                        if done_pro == TILES:
                            allgather(l + 1, 1)

    nc.compile()
    return nc


def _wrap_idx(idx_flat):
    """int16 idx vector -> [128, len/16] wrapped (16-partition) layout."""
    n = idx_flat.shape[0]
    assert n % 16 == 0
    w = idx_flat.reshape(n // 16, 16).T            # [16, n/16]
    return np.tile(w, (8, 1)).astype(np.int16)     # [128, n/16]


def _balance(deg):
    """Greedy bin-pack NP_ALL nodes into 392 buckets of exactly 128 slots,
    equalizing per-bucket edge counts. Returns (core_of, loc_of)."""
    import heapq

    nb = CORES * TILES
    order = np.argsort(-deg, kind="stable")
    heap = [(0, b) for b in range(nb)]
    heapq.heapify(heap)
    slots_used = np.zeros(nb, np.int32)
    core_of = np.empty(NP_ALL, np.int32)
    loc_of = np.empty(NP_ALL, np.int32)
    for n in order:
        e, b = heapq.heappop(heap)
        core_of[n] = b // TILES
        loc_of[n] = (b % TILES) * 128 + slots_used[b]
        slots_used[b] += 1
        if slots_used[b] < 128:
            heapq.heappush(heap, (e + int(deg[n]), b))
    return core_of, loc_of


def _prep(inputs):
    x = np.asarray(inputs["x"], np.float32)
    ei = np.asarray(inputs["edge_index"]).astype(np.int64)
    Wl0 = np.asarray(inputs["Wl0"], np.float32)
    Wr0 = np.asarray(inputs["Wr0"], np.float32)
    bl0 = np.asarray(inputs["bl0"], np.float32)
    br0 = np.asarray(inputs["br0"], np.float32)
    Wl = np.asarray(inputs["Wl"], np.float32)
    Wr = np.asarray(inputs["Wr"], np.float32)
    bl = np.asarray(inputs["bl"], np.float32)
    br = np.asarray(inputs["br"], np.float32)
    att = np.asarray(inputs["att"], np.float32)
    bias = np.asarray(inputs["bias"], np.float32)

    # --- attention folding: per layer, permute columns so a >= 0 comes
    # first (p_l columns) and scale columns by max(|a|, eps). ---
    perms, scales, p_cnt = [], [], []
    for l in range(T):
        a = att[l]
        pos = a >= 0
        perm = np.argsort(~pos, kind="stable")
        perms.append(perm)
        p_cnt.append(int(pos.sum()))
        scales.append(np.maximum(np.abs(a[perm]), 1e-12))

    # effective weights: fold perm+scale on output cols, and the inverse
    # of the previous layer's fold on the input rows
    Wl_eff, Wr_eff = [], []
    for l in range(T):
        wl_ = Wl0 if l == 0 else Wl[l - 1]
        wr_ = Wr0 if l == 0 else Wr[l - 1]
        if l > 0:
            unscale = (1.0 / scales[l - 1])[:, None]
            wl_ = wl_[perms[l - 1], :] * unscale
            wr_ = wr_[perms[l - 1], :] * unscale
        Wl_eff.append(wl_[:, perms[l]] * scales[l][None, :])
        Wr_eff.append(wr_[:, perms[l]] * scales[l][None, :])

    # xr bias (score side): (bl + br) folded
    brow2 = np.stack([
        ((bl0 + br0) if l == 0 else (bl[l - 1] + br[l - 1]))[perms[l]]
        * scales[l]
        for l in range(T)])
    # relu-step bias for inner layers: (bias_l + bl_l) folded
    bo_rows = []
    for l in range(T):
        b_ = bias[l] + (bl0 if l == 0 else bl[l - 1])
        bo_rows.append(b_[perms[l]] * scales[l])
    bout = np.stack(bo_rows).T.copy()   # [D, T]; col T-1 unused on-chip
    # host-side final unfold
    fin_perm = perms[T - 1]
    fin_scale = scales[T - 1]
    fin_bias = bias[T - 1] + bl[T - 2]

    # nodes 0..N-1 real, N..NP_ALL-1 virtual pads (degree-1 self loops keep
    # their softmax denominators finite)
    loop = np.arange(NP_ALL, dtype=np.int64)
    src = np.concatenate([ei[0], loop])
    dst = np.concatenate([ei[1], loop])

    deg = np.bincount(dst, minlength=NP_ALL)
    core_of, loc_of = _balance(deg)

    owner = core_of[dst].astype(np.int64)
    local = loc_of[dst].astype(np.int64)       # 0..SHP-1
    sc = core_of[src].astype(np.int64)
    sl = loc_of[src].astype(np.int64)
    s_half = (sl >= H1).astype(np.int64)
    g_row = np.where(s_half == 0, sc * H1 + sl, sc * H2 + (sl - H1))

    tile_of = local >> 7
    cnt = np.zeros((CORES, TILES, 2), np.int64)
    np.add.at(cnt, (owner, tile_of, s_half), 1)

    K1s = tuple(int(v) for v in
                np.ceil(cnt[:, :, 0].max(axis=0) / 128).astype(np.int64))
    K2s = tuple(int(v) for v in
                np.ceil(cnt[:, :, 1].max(axis=0) / 128).astype(np.int64))

    groups = [(t, t + 1) for t in range(0, TILES - 1, 2)] + [(TILES - 1,)]
    CH = sum(K1s) + sum(K2s)

    # per-core packing
    ixls, ohs, ohEs = [], [], []
    for c in range(CORES):
        sel = owner == c
        e_tile = tile_of[sel]
        e_half = s_half[sel]
        e_g = g_row[sel]
        e_dloc = local[sel]

        order = np.lexsort((e_dloc, e_half, e_tile))
        e_tile, e_half, e_g, e_dloc = (
            e_tile[order], e_half[order], e_g[order], e_dloc[order])
        bounds = np.searchsorted(
            e_tile * 2 + e_half, np.arange(TILES * 2 + 1))

        ixl = np.zeros(CH * 128, np.int64)
        oh = np.zeros((128, CH * 128), np.float32)
        ohE = np.zeros((128, CH * 128), np.float32)
        pos = 0
        for ts in groups:
            for half, Ks in ((0, K1s), (1, K2s)):
                for t in ts:
                    kk = Ks[t]
                    b0, b1 = bounds[t * 2 + half], bounds[t * 2 + half + 1]
                    ne = b1 - b0
                    assert ne <= kk * 128
                    sl2 = slice(pos * 128, pos * 128 + ne)
                    ixl[sl2] = e_g[b0:b1]
                    i_in = np.arange(ne)
                    dloc7 = e_dloc[b0:b1] & 127
                    oh[dloc7, (pos + (i_in >> 7)) * 128 + (i_in & 127)] = 1.0
                    ohE[i_in & 127, (pos + (i_in >> 7)) * 128 + dloc7] = 1.0
                    pos += kk
        assert pos == CH
        assert ixl.max() < 32768
        ixls.append(_wrap_idx(ixl.astype(np.int16)))
        ohs.append(oh)
        ohEs.append(ohE)

    def bf16(a):
        import jax.numpy as jnp
        return np.asarray(jnp.asarray(np.asarray(a, np.float32),
                                      dtype=jnp.bfloat16))

    Wlr0 = bf16(np.concatenate([Wl_eff[0], Wr_eff[0]], axis=1))
    Wlr_ = bf16(np.stack([
        np.concatenate([Wl_eff[l], Wr_eff[l]], axis=1)
        for l in range(1, T)]))
    common = dict(Wlr0=Wlr0, Wlr=Wlr_, br2=brow2.astype(np.float32),
                  bout=bout.astype(np.float32))
    in_maps = []
    nodes = np.arange(N)
    for c in range(CORES):
        xT_own = np.zeros((DIN, SHP), np.float32)
        m = core_of[:N] == c
        xT_own[:, loc_of[:N][m]] = x[nodes[m]].T
        in_maps.append(dict(common, xT_own=bf16(xT_own), ixl=ixls[c],
                            hs=bf16(ohs[c]), he=bf16(ohEs[c])))
    params = (K1s, K2s, tuple(p_cnt))
    fin = (fin_perm, fin_scale, fin_bias)
    return params, in_maps, (core_of, loc_of), fin


_CACHE = {}


def kernel(**inputs) -> np.ndarray:
    out, _ = _run(inputs)
    return out


def _run(inputs, **kw):
    params, in_maps, (core_of, loc_of), fin = _prep(inputs)
    if params not in _CACHE:
        _CACHE[params] = _build_nc(params)
    nc = _CACHE[params]
    res = run_bass_kernel_spmd(nc, in_maps, core_ids=list(range(CORES)), **kw)
    outs = np.stack([res.results[c]["out"] for c in range(CORES)])
    out = outs[core_of[:N], loc_of[:N]]
    # undo final-layer attention fold: unpermute + unscale cols, add bias
    fin_perm, fin_scale, fin_bias = fin
    res_f = np.empty_like(out)
    res_f[:, fin_perm] = out / fin_scale[None, :]
    res_f += fin_bias[None, :]
    return res_f.astype(np.float32), res
